# revision 7
# baseline (speedup 1.0000x reference)
"""Trainium2 Bass kernel for DEIM multi-scale deformable attention.

Strategy:
  - Data-parallel over batch: 16 batches -> 8 cores, 2 batches/core.
  - Per (b,q,level): ONE 4x4-pixel x 256-channel bf16 window gather
    (4 descriptors of 2KB) instead of per-corner gathers; exact bilinear
    hat weights vs the window grid reproduce grid_sample(zeros) exactly.
  - M[h, iy, jx] = sum_p attn[h,p]*haty[h,p,iy]*hatx[h,p,jx] folds softmax
    attention + bilinear interp into a 16-pixel stencil per head.
  - Channels are HOST-permuted to (d, h) order (head index innermost) so the
    window multiply's broadcast of M over the 32 head-dims has a unit-stride
    innermost run of 8 -> DVE 2x_1p bf16 mode (the f32 broadcast-stride-0
    version ran at 1x). The whole window path (gather, multiply, pixel-sum
    tree, output projection operands) is bf16; f32 elsewhere.
  - Pixel-sum done as a 4-stage pairwise tree of bf16 tensor_tensor adds
    (2x mode) instead of a 1x tensor_reduce.
  - Engine split: DVE = window multiply + tree (+ softmax/geometry one-
    shots); POOL = gathers (4 SWDGE queues, overlapped transfers), stencil
    build (prod/mm/me) ; ACT = hats, PSUM evacuations; PE = projections.
"""

import os
from contextlib import ExitStack

import numpy as np

# ---------------------------------------------------------------------------
# Problem constants (hardcoded per harness contract)
# ---------------------------------------------------------------------------
B, Q, C, NH, NP, NL = 16, 300, 256, 8, 4, 4
HD = C // NH
SPATIAL = ((80, 80), (40, 40), (20, 20), (30, 70))  # (h, w) per level
S = sum(h * w for h, w in SPATIAL)  # 10500
BASE_L = [0, 6400, 8000, 8400]
H_L = [h for h, w in SPATIAL]
W_L = [w for h, w in SPATIAL]

NCORES = 8
BPC = B // NCORES          # batches per core
QS = BPC * Q               # query slots per core (600)
QSP = 640                  # padded query slots (5 x 128)
QT_SIZES = [128, 128, 128, 128, 128]  # padded; rows past 600 are garbage
NQT = 5
MEMROWS = BPC * S          # 21000 pixel rows per core
WIN = 4                    # window size (pixels per axis)
ELEM = WIN * C             # gather element: 4 pixels x 256 ch


def _build_program():
    import concourse.bacc as bacc
    import concourse.bass as bass
    import concourse.tile as tile
    from concourse import mybir
    from concourse.masks import make_identity

    f32 = mybir.dt.float32
    bf16 = mybir.dt.bfloat16
    i16 = mybir.dt.int16

    nc = bacc.Bacc("TRN2", target_bir_lowering=False, debug=False,
                   num_devices=NCORES, num_swdge_queues=4)

    AF = mybir.ActivationFunctionType
    OP = mybir.AluOpType

    def ap_of(t, off, pairs):
        """Manual access pattern: offset in elements relative to t's own
        offset; pairs = [[step, count], ...] (partition first; partition
        step rescaled to the tensor's per-partition stride)."""
        a = t[:] if hasattr(t, "__getitem__") else t
        pairs = [list(p) for p in pairs]
        if a.space == bass.MemorySpace.SBUF:
            pairs[0][0] *= a.ap[0][0]
        return bass.AP(tensor=a.tensor, offset=a.offset + off, ap=pairs)

    # ------------------------------------------------------------------
    # DRAM I/O  (mem/qT/weights in bf16; channel order is (d, h))
    # ------------------------------------------------------------------
    memd = nc.dram_tensor("mem", [MEMROWS, C], bf16, kind="ExternalInput")
    qTd = nc.dram_tensor("qT", [C, QS], bf16, kind="ExternalInput")
    refd = nc.dram_tensor("refpix", [QSP, 2 * NL], f32, kind="ExternalInput")
    woffd = nc.dram_tensor("Woff", [C, 256], bf16, kind="ExternalInput")
    wattnd = nc.dram_tensor("Wattn", [C, NH * NL * NP], bf16, kind="ExternalInput")
    woutd = nc.dram_tensor("Wout", [C, C], bf16, kind="ExternalInput")
    boutd = nc.dram_tensor("bout", [1, C], bf16, kind="ExternalInput")
    outd = nc.dram_tensor("out", [QS, C], f32, kind="ExternalOutput")

    with tile.TileContext(nc) as tc, ExitStack() as ctx:
        dram = ctx.enter_context(tc.tile_pool(name="dram", bufs=1, space="DRAM"))
        idxd = dram.tile([NQT, 4 * 512], i16)

        singles = ctx.enter_context(tc.tile_pool(name="singles", bufs=1))
        psum_mm = ctx.enter_context(tc.tile_pool(name="psum_mm", bufs=2, space="PSUM"))
        psum_tr = ctx.enter_context(tc.tile_pool(name="psum_tr", bufs=2, space="PSUM"))
        psum_o = ctx.enter_context(tc.tile_pool(name="psum_o", bufs=2, space="PSUM"))
        work = ctx.enter_context(tc.tile_pool(name="work", bufs=2))
        winp = ctx.enter_context(tc.tile_pool(name="winp", bufs=4))

        # ---------------- one-time constants ----------------
        sb_qT = singles.tile([128, 2, QS], bf16)
        nc.sync.dma_start(out=sb_qT, in_=qTd.ap().rearrange("(k p) q -> p k q", p=128))
        sb_Woff = singles.tile([128, 2, 256], bf16)
        nc.sync.dma_start(out=sb_Woff, in_=woffd.ap().rearrange("(k p) n -> p k n", p=128))
        sb_Wattn = singles.tile([128, 2, 128], bf16)
        nc.sync.dma_start(out=sb_Wattn, in_=wattnd.ap().rearrange("(k p) n -> p k n", p=128))
        sb_Wout = singles.tile([128, 2, 256], bf16)
        nc.sync.dma_start(out=sb_Wout, in_=woutd.ap().rearrange("(k p) n -> p k n", p=128))
        sb_bout = singles.tile([1, 256], bf16)
        nc.sync.dma_start(out=sb_bout, in_=boutd.ap())
        sb_ones = singles.tile([1, 128], bf16)
        nc.vector.memset(sb_ones, 1.0)
        ident = singles.tile([128, 128], bf16)
        make_identity(nc, ident[:])

        # clip-hi per (l, xy): xy=0 -> w-4, xy=1 -> h-4
        wh4 = singles.tile([128, NL, 2], f32)
        for l in range(NL):
            nc.vector.memset(wh4[:, l, 0:1], float(W_L[l] - WIN))
            nc.vector.memset(wh4[:, l, 1:2], float(H_L[l] - WIN))
        wrow = singles.tile([128, NL], f32)
        for l in range(NL):
            nc.vector.memset(wrow[:, l:l + 1], float(W_L[l]))
        jw = singles.tile([128, NL, WIN], f32)
        for l in range(NL):
            for j in range(WIN):
                nc.vector.memset(jw[:, l, j:j + 1], float(j * W_L[l]))
        jneg = singles.tile([128, WIN], f32)
        for j in range(WIN):
            nc.vector.memset(jneg[:, j:j + 1], float(-j))
        # per-qt level base (batch offset included): partition p of tile it
        # holds (q0+p)//Q * S + BASE_L[l].
        baselv = singles.tile([128, NQT, NL], f32)
        for it in range(NQT):
            q0 = it * 128
            for l in range(NL):
                nc.vector.memset(baselv[:, it, l:l + 1], float(BASE_L[l]))
                if q0 + 127 >= Q and q0 < Q:
                    nc.gpsimd.affine_select(
                        out=baselv[:, it, l:l + 1],
                        in_=baselv[:, it, l:l + 1],
                        pattern=[[0, 1]], base=Q - 1 - q0,
                        channel_multiplier=-1,
                        compare_op=mybir.AluOpType.is_ge,
                        fill=float(S + BASE_L[l]))
                elif q0 >= Q:
                    nc.vector.memset(baselv[:, it, l:l + 1],
                                     float(S + BASE_L[l]))

        # ---------------- one-shot geometry for ALL tiles ----------------
        # refp[q, t, (l,xy)] ; all geometry in [128, NQT, 8]
        geo = singles.tile([128, 4, NQT, 2 * NL], f32)
        refp = geo[:, 0]
        xsc = geo[:, 1]
        pxm = geo[:, 2]
        vb = geo[:, 3]
        nc.sync.dma_start(out=refp,
                          in_=refd.ap().rearrange("(t p) x -> p t x", p=128))
        MAGIC = float(1 << 23)
        # floor(refp) via magic-add; vb-MAGIC-1, clamp into [0, wh-4]
        nc.vector.tensor_scalar(vb, refp, 0.5, MAGIC, OP.subtract, OP.add)
        nc.vector.tensor_scalar(xsc, vb, MAGIC + 1.0, 0.0, OP.subtract, OP.max)
        nc.vector.tensor_tensor(
            xsc[:, :, :], xsc[:, :, :],
            ap_of(wh4, 0, [[1, 128], [0, NQT], [1, 2 * NL]]), op=OP.min)
        nc.vector.tensor_sub(pxm, refp, xsc)
        # P0 = ysc*w + xsc + base; idx = P0 + j*w
        p0t = singles.tile([128, NQT, NL], f32)
        nc.vector.tensor_mul(
            p0t[:, :, :],
            ap_of(geo, 1 * NQT * 8 + 1, [[1, 128], [8, NQT], [2, NL]]),  # y of xsc
            ap_of(wrow, 0, [[1, 128], [0, NQT], [1, NL]]))
        nc.vector.tensor_add(
            p0t[:, :, :], p0t[:, :, :],
            ap_of(geo, 1 * NQT * 8, [[1, 128], [8, NQT], [2, NL]]))       # x of xsc
        nc.vector.tensor_add(p0t[:, :, :], p0t[:, :, :], baselv[:, :, :])
        idxf = singles.tile([128, NQT, NL, WIN], f32)
        nc.vector.tensor_add(
            idxf[:, :, :, :],
            ap_of(p0t, 0, [[1, 128], [NL, NQT], [1, NL], [0, WIN]]),
            ap_of(jw, 0, [[1, 128], [0, NQT], [WIN, NL], [1, WIN]]))
        idxi = singles.tile([128, NQT, NL * WIN], i16)
        nc.vector.tensor_copy(
            idxi[:, :, :],
            idxf[:, :, :, :].rearrange("q t l j -> q t (l j)"))
        for it in range(NQT):
            nc.sync.dma_start(
                out=ap_of(idxd[it:it + 1, :], 0, [[1, 128], [512, NL], [128, WIN]]),
                in_=idxi[:, it, :])
        idxw = singles.tile([128, NQT, NL, 32], i16)
        for it in range(NQT):
            for g in range(8):
                nc.sync.dma_start(
                    out=idxw[16 * g:16 * (g + 1), it, :, :],
                    in_=ap_of(idxd[it:it + 1, :], 0,
                              [[1, 16], [512, NL], [16, 32]]))

        # ---------------- per-tile projections -> offs/elog ----------------
        offs_all = singles.tile([128, NQT, 256], f32)
        elog_all = singles.tile([128, NQT, 128], f32)
        # pad rows (600..639 -> partitions 88..127 of the last tile) never
        # get projection results; define them so downstream one-shots read
        # finite values (partition offsets must be 32-aligned; rows 64..87
        # are overwritten by the real copies afterwards).
        nc.vector.memset(offs_all[64:128, NQT - 1, :], 0.0)
        nc.vector.memset(elog_all[64:128, NQT - 1, :], 1.0)
        for it in range(NQT):
            q0 = it * 128
            qlen = min(QS - q0, 128)
            ql = slice(0, qlen)
            ps_off = psum_mm.tile([128, 256], f32, tag="ps_off")
            nc.tensor.matmul(ps_off[ql, :], lhsT=sb_qT[:, 0, q0:q0 + qlen],
                             rhs=sb_Woff[:, 0, :], start=True, stop=False)
            nc.tensor.matmul(ps_off[ql, :], lhsT=sb_qT[:, 1, q0:q0 + qlen],
                             rhs=sb_Woff[:, 1, :], start=False, stop=True)
            ps_log = psum_mm.tile([128, 128], f32, tag="ps_log")
            nc.tensor.matmul(ps_log[ql, :], lhsT=sb_qT[:, 0, q0:q0 + qlen],
                             rhs=sb_Wattn[:, 0, :], start=True, stop=False)
            nc.tensor.matmul(ps_log[ql, :], lhsT=sb_qT[:, 1, q0:q0 + qlen],
                             rhs=sb_Wattn[:, 1, :], start=False, stop=True)
            nc.scalar.copy(offs_all[ql, it, :], ps_off[ql, :])
            nc.scalar.activation(elog_all[ql, it, :], ps_log[ql, :], AF.Exp)

        # ---------------- one-shot softmax over (l,p) per h ----------------
        ssum = singles.tile([128, NQT, NH], f32)
        nc.vector.tensor_reduce(
            ssum[:, :, :],
            elog_all[:, :, :].rearrange("q t (h s) -> q t h s", h=NH),
            axis=mybir.AxisListType.X, op=OP.add)
        rinv = singles.tile([128, NQT, NH], f32)
        nc.vector.reciprocal(rinv[:, :, :].rearrange("q t h -> q (t h)"),
                             ssum[:, :, :].rearrange("q t h -> q (t h)"))
        # attnR[q, t, (l,h,p)] = elog[q, t, (h,l,p)] * rinv[q, t, h]
        attnR = singles.tile([128, NQT, 128], f32)
        for it in range(NQT):
            nc.vector.tensor_mul(
                attnR[:, it, :],
                ap_of(elog_all, it * 128, [[1, 128], [4, NL], [16, NH], [1, NP]]),
                ap_of(rinv, it * NH, [[1, 128], [0, NL], [1, NH], [0, NP]]))

        # ---------------- one-shot U and hats ----------------
        # U[q, t, l, xy, hp] = offs[q, t, (l,h,p,xy)] + pxm[q, t, (l,xy)]
        uu = singles.tile([128, NQT, NL, 2, 32], f32)
        for it in range(NQT):
            nc.gpsimd.tensor_add(
                uu[:, it, :, :, :],
                ap_of(offs_all, it * 256, [[1, 128], [64, NL], [1, 2], [2, 32]]),
                ap_of(geo, 2 * NQT * 8 + it * 8, [[1, 128], [2, NL], [1, 2], [0, 32]]))
        # hat[q, t, j, l, xy, hp] = relu(1 - |U - j|)
        hat = singles.tile([128, NQT, WIN, NL, 2, 32], f32)
        for it in range(NQT):
            for j in range(WIN):
                nc.scalar.activation(hat[:, it, j, :, :, :],
                                     uu[:, it, :, :, :], AF.Abs,
                                     bias=jneg[:, j:j + 1])
        nc.scalar.activation(
            hat[:, :, :, :, :, :].rearrange("q t j l x s -> q (t j l x s)"),
            hat[:, :, :, :, :, :].rearrange("q t j l x s -> q (t j l x s)"),
            AF.Relu, bias=1.0, scale=-1.0)
        # AFY[q, t, (l,h,p,i)] = attnR[q,t,(l,h,p)] * haty[q,t,(i,l,hp)]
        afy = singles.tile([128, NQT, NL, 8, NP, WIN], f32)
        for it in range(NQT):
            nc.vector.tensor_mul(
                afy[:, it, :, :, :, :],
                ap_of(hat, it * 1024 + 32, [[1, 128], [64, NL], [1, 32], [256, WIN]]),
                ap_of(attnR, it * 128, [[1, 128], [32, NL], [1, 32], [0, WIN]]))

        # ---------------- main loop: gather + stencil + reduce ----------------
        for it in range(NQT):
            q0 = it * 128
            qlen = QT_SIZES[it]
            ql = slice(0, qlen)
            res4 = work.tile([128, NL, 256], bf16, tag="res4")
            for l in range(NL):
                win = winp.tile([128, WIN, ELEM], bf16, tag="win")
                nc.gpsimd.dma_gather(
                    out_ap=win[:, :, :],
                    in_ap=ap_of(memd.ap(), 0, [[C, MEMROWS - (WIN - 1)], [1, ELEM]]),
                    idxs_ap=idxw[:, it, l, :],
                    num_idxs=512, num_idxs_reg=512,
                    elem_size=ELEM, elem_step=C,
                    queue_num=l % 4)

                # prod[q, (h,i,j), p] = afy[q,t,(l,h,p,i)] * hatx[q,t,(j,l,hp)]
                prod = work.tile([128, 8 * WIN * WIN, NP], f32, tag="prod")
                for p in range(NP):
                    nc.gpsimd.tensor_mul(
                        ap_of(prod, p, [[1, qlen], [NP, 8 * WIN * WIN]]),
                        ap_of(afy, it * 512 + l * 128 + p * WIN,
                              [[1, qlen], [16, 8], [1, WIN], [0, WIN]]),
                        ap_of(hat, it * 1024 + l * 64 + p,
                              [[1, qlen], [4, 8], [0, WIN], [256, WIN]]))
                mmh = work.tile([128, 8 * WIN * WIN, 2], f32, tag="mmh")
                nc.gpsimd.tensor_add(
                    mmh[ql, :, :],
                    ap_of(prod, 0, [[1, qlen], [NP, 8 * WIN * WIN], [1, 2]]),
                    ap_of(prod, 2, [[1, qlen], [NP, 8 * WIN * WIN], [1, 2]]))
                mm = work.tile([128, 8, WIN, WIN], f32, tag="mm")
                nc.gpsimd.tensor_add(
                    mm[ql, :, :, :].rearrange("q h i j -> q (h i j)"),
                    ap_of(mmh, 0, [[1, qlen], [2, 8 * WIN * WIN]]),
                    ap_of(mmh, 1, [[1, qlen], [2, 8 * WIN * WIN]]))
                # ME16[q, (i,j,h)] = mm[q, (h,i,j)]  (bf16, head innermost)
                me16 = work.tile([128, WIN, WIN, 8], bf16, tag="me16")
                nc.gpsimd.tensor_copy(
                    me16[ql, :, :, :],
                    ap_of(mm, 0, [[1, qlen], [4, WIN], [1, WIN], [16, 8]]))
                # winM[q, px, d, h] = win[q, px, (d,h)] * ME16[q, px, h]
                # inner run of 8 (head dim) unit-stride -> DVE 2x bf16
                winM = work.tile([128, 4096], bf16, tag="winM")
                nc.vector.tensor_mul(
                    ap_of(winM, 0, [[1, qlen], [256, 16], [8, 32], [1, 8]]),
                    ap_of(win, 0, [[1, qlen], [256, 16], [8, 32], [1, 8]]),
                    ap_of(me16, 0, [[1, qlen], [8, 16], [0, 32], [1, 8]]))
                # pixel-sum: 4-stage pairwise tree, all bf16 2x adds
                t1 = work.tile([128, 2048], bf16, tag="t1")
                nc.vector.tensor_add(t1[ql, :], winM[ql, 0:2048],
                                     winM[ql, 2048:4096])
                t2 = work.tile([128, 1024], bf16, tag="t2")
                nc.vector.tensor_add(t2[ql, :], t1[ql, 0:1024], t1[ql, 1024:2048])
                t3 = work.tile([128, 512], bf16, tag="t3")
                nc.vector.tensor_add(t3[ql, :], t2[ql, 0:512], t2[ql, 512:1024])
                nc.vector.tensor_add(res4[ql, l, :], t3[ql, 0:256], t3[ql, 256:512])

            # sum over levels (tree, bf16)
            nc.vector.tensor_add(res4[ql, 0:2, :], res4[ql, 0:2, :], res4[ql, 2:4, :])
            res = work.tile([128, 256], bf16, tag="res")
            nc.vector.tensor_add(res[ql, :], res4[ql, 0, :], res4[ql, 1, :])

            # --- output projection: out = res @ Wout + bout (bf16 PE)
            resT = work.tile([128, 2, 128], bf16, tag="resT")
            for hh in range(2):
                ps_t = psum_tr.tile([128, 128], bf16, tag="ps_t")
                nc.tensor.transpose(ps_t[:, ql], res[ql, 128 * hh:128 * (hh + 1)],
                                    ident[ql, ql])
                nc.scalar.copy(resT[:, hh, ql], ps_t[:, ql])
            ps_out = psum_o.tile([128, 256], f32, tag="ps_out")
            nc.tensor.matmul(ps_out[ql, :], lhsT=resT[:, 0, ql],
                             rhs=sb_Wout[:, 0, :], start=True, stop=False)
            nc.tensor.matmul(ps_out[ql, :], lhsT=resT[:, 1, ql],
                             rhs=sb_Wout[:, 1, :], start=False, stop=False)
            nc.tensor.matmul(ps_out[ql, :], lhsT=sb_ones[0:1, ql],
                             rhs=sb_bout[0:1, :], start=False, stop=True)
            outt = work.tile([128, 256], f32, tag="outt")
            nc.scalar.copy(outt[ql, :], ps_out[ql, :])
            qlen_out = min(QS - q0, 128)
            nc.sync.dma_start(out=outd.ap()[q0:q0 + qlen_out, :],
                              in_=outt[0:qlen_out, :])

    nc.compile()
    return nc


_NC_CACHE = {}
LAST_RESULTS = None


def _get_nc():
    if "nc" not in _NC_CACHE:
        _NC_CACHE["nc"] = _build_program()
    return _NC_CACHE["nc"]


def host_prep(query, memory, ref_points, W_off, b_off, W_attn, b_attn,
              W_out, b_out):
    """Build the 8 per-core input maps (pure layout/dtype transforms)."""
    import ml_dtypes
    bf16 = ml_dtypes.bfloat16

    query = np.ascontiguousarray(query, dtype=np.float32)
    memory = np.ascontiguousarray(memory, dtype=np.float32)
    ref = np.asarray(ref_points, dtype=np.float32)
    W_off = np.asarray(W_off, dtype=np.float32)
    b_off = np.asarray(b_off, dtype=np.float32)
    W_attn = np.asarray(W_attn, dtype=np.float32)
    b_attn = np.asarray(b_attn, dtype=np.float32)
    assert np.all(b_off == 0.0) and np.all(b_attn == 0.0), \
        "nonzero offset/attn biases not folded on device"
    # W_off cols (h,l,p,xy) -> (l,h,p,xy)
    Woff_r = np.ascontiguousarray(
        W_off.reshape(C, NH, NL, NP, 2).transpose(0, 2, 1, 3, 4)
        .reshape(C, 256)).astype(bf16)
    Wattn_r = np.ascontiguousarray(W_attn).astype(bf16)
    # W_out rows permuted to the (d, h) channel order the window path uses
    Wout_r = np.ascontiguousarray(
        np.asarray(W_out, np.float32).reshape(NH, HD, C)
        .transpose(1, 0, 2).reshape(C, C)).astype(bf16)
    bout = np.ascontiguousarray(
        np.asarray(b_out, dtype=np.float32).reshape(1, C)).astype(bf16)

    wh = np.array([[w, h] for h, w in SPATIAL], dtype=np.float32)
    in_maps = []
    for c in range(NCORES):
        bs = slice(BPC * c, BPC * (c + 1))
        qT = np.ascontiguousarray(
            query[bs].reshape(QS, C).T).astype(bf16)          # [256, 600]
        # memory channels (h,d) -> (d,h), bf16
        mem = np.ascontiguousarray(
            memory[bs].reshape(MEMROWS, NH, HD).transpose(0, 2, 1)
            .reshape(MEMROWS, C)).astype(bf16)
        refc = ref[bs].reshape(QS, NL, 2)
        refpix = refc * wh[None, :, :] - 0.5                   # [600, l, xy]
        refpix = refpix.reshape(QS, 2 * NL).astype(np.float32)
        refpad = np.full((QSP, 2 * NL), 5.0, np.float32)       # safe interior
        refpad[:QS] = refpix
        in_maps.append(dict(mem=mem, qT=qT, refpix=np.ascontiguousarray(refpad),
                            Woff=Woff_r, Wattn=Wattn_r, Wout=Wout_r, bout=bout))
    return in_maps


def kernel(**inputs):
    global LAST_RESULTS
    from concourse.bass_utils import run_bass_kernel_spmd

    nc = _get_nc()
    in_maps = host_prep(**inputs)
    trace = bool(int(os.environ.get("KERNEL_TRACE", "0")))
    res = run_bass_kernel_spmd(nc, in_maps, core_ids=list(range(NCORES)),
                               trace=trace)
    LAST_RESULTS = res
    out = np.empty((B, Q, C), dtype=np.float32)
    for c in range(NCORES):
        out[BPC * c:BPC * (c + 1)] = res.results[c]["out"].reshape(BPC, Q, C)
    return out


# revision 12
# speedup vs baseline: 1.1066x; 1.1066x over previous
"""Trainium2 Bass kernel for DEIM multi-scale deformable attention.

Strategy:
  - Data-parallel over batch: 16 batches -> 8 cores, 2 batches/core.
  - Per (b,q,level): ONE 4x4-pixel x 256-channel bf16 window gather
    (4 descriptors of 2KB) instead of per-corner gathers; exact bilinear
    hat weights vs the window grid reproduce grid_sample(zeros) exactly.
  - M[h, iy, jx] = sum_p attn[h,p]*haty[h,p,iy]*hatx[h,p,jx] folds softmax
    attention + bilinear interp into a 16-pixel stencil per head.
  - Channels are HOST-permuted to (d, h) order (head index innermost) so the
    window multiply's broadcast of M over the 32 head-dims has a unit-stride
    innermost run of 8 -> DVE 2x_1p bf16 mode (the f32 broadcast-stride-0
    version ran at 1x). The whole window path (gather, multiply, pixel-sum
    tree, output projection operands) is bf16; f32 elsewhere.
  - Pixel-sum done as a 4-stage pairwise tree of bf16 tensor_tensor adds
    (2x mode) instead of a 1x tensor_reduce.
  - Engine split: DVE = window multiply + tree (+ softmax/geometry one-
    shots); POOL = gathers (4 SWDGE queues, overlapped transfers), stencil
    build (prod/mm/me) ; ACT = hats, PSUM evacuations; PE = projections.
"""

import os
from contextlib import ExitStack

import numpy as np

# ---------------------------------------------------------------------------
# Problem constants (hardcoded per harness contract)
# ---------------------------------------------------------------------------
B, Q, C, NH, NP, NL = 16, 300, 256, 8, 4, 4
HD = C // NH
SPATIAL = ((80, 80), (40, 40), (20, 20), (30, 70))  # (h, w) per level
S = sum(h * w for h, w in SPATIAL)  # 10500
BASE_L = [0, 6400, 8000, 8400]
H_L = [h for h, w in SPATIAL]
W_L = [w for h, w in SPATIAL]

NCORES = 8
BPC = B // NCORES          # batches per core
QS = BPC * Q               # query slots per core (600)
QSP = 640                  # padded query slots (5 x 128)
QT_SIZES = [128, 128, 128, 128, 128]  # padded; rows past 600 are garbage
NQT = 5
MEMROWS = BPC * S          # 21000 pixel rows per core
WIN = 4                    # window size (pixels per axis)
ELEM = WIN * C             # gather element: 4 pixels x 256 ch


def _build_program():
    import concourse.bacc as bacc
    import concourse.bass as bass
    import concourse.tile as tile
    from concourse import mybir
    from concourse.masks import make_identity

    f32 = mybir.dt.float32
    bf16 = mybir.dt.bfloat16
    i16 = mybir.dt.int16

    nc = bacc.Bacc("TRN2", target_bir_lowering=False, debug=False,
                   num_devices=NCORES, num_swdge_queues=4)

    AF = mybir.ActivationFunctionType
    OP = mybir.AluOpType

    def ap_of(t, off, pairs):
        """Manual access pattern: offset in elements relative to t's own
        offset; pairs = [[step, count], ...] (partition first; partition
        step rescaled to the tensor's per-partition stride)."""
        a = t[:] if hasattr(t, "__getitem__") else t
        pairs = [list(p) for p in pairs]
        if a.space == bass.MemorySpace.SBUF:
            pairs[0][0] *= a.ap[0][0]
        return bass.AP(tensor=a.tensor, offset=a.offset + off, ap=pairs)

    # ------------------------------------------------------------------
    # DRAM I/O  (mem/qT/weights in bf16; channel order is (d, h))
    # ------------------------------------------------------------------
    memd = nc.dram_tensor("mem", [MEMROWS, C], bf16, kind="ExternalInput")
    qTd = nc.dram_tensor("qT", [C, QS], bf16, kind="ExternalInput")
    refd = nc.dram_tensor("refpix", [QSP, 2 * NL], f32, kind="ExternalInput")
    woffd = nc.dram_tensor("Woff", [C, 256], bf16, kind="ExternalInput")
    wattnd = nc.dram_tensor("Wattn", [C, NH * NL * NP], bf16, kind="ExternalInput")
    woutd = nc.dram_tensor("Wout", [C, C], bf16, kind="ExternalInput")
    boutd = nc.dram_tensor("bout", [1, C], bf16, kind="ExternalInput")
    outd = nc.dram_tensor("out", [QS, C], f32, kind="ExternalOutput")

    with tile.TileContext(nc) as tc, ExitStack() as ctx:
        dram = ctx.enter_context(tc.tile_pool(name="dram", bufs=1, space="DRAM"))
        idxd = dram.tile([NQT, 4 * 512], i16)

        singles = ctx.enter_context(tc.tile_pool(name="singles", bufs=1))
        psum_mm = ctx.enter_context(tc.tile_pool(name="psum_mm", bufs=2, space="PSUM"))
        psum_tr = ctx.enter_context(tc.tile_pool(name="psum_tr", bufs=2, space="PSUM"))
        psum_o = ctx.enter_context(tc.tile_pool(name="psum_o", bufs=2, space="PSUM"))
        work = ctx.enter_context(tc.tile_pool(name="work", bufs=3))
        winp = ctx.enter_context(tc.tile_pool(name="winp", bufs=4))

        # ---------------- one-time constants ----------------
        sb_qT = singles.tile([128, 2, QS], bf16)
        nc.sync.dma_start(out=sb_qT, in_=qTd.ap().rearrange("(k p) q -> p k q", p=128))
        sb_Woff = singles.tile([128, 2, 256], bf16)
        nc.scalar.dma_start(out=sb_Woff, in_=woffd.ap().rearrange("(k p) n -> p k n", p=128))
        sb_Wattn = singles.tile([128, 2, 128], bf16)
        nc.scalar.dma_start(out=sb_Wattn, in_=wattnd.ap().rearrange("(k p) n -> p k n", p=128))
        sb_Wout = singles.tile([128, 2, 256], bf16)
        nc.scalar.dma_start(out=sb_Wout, in_=woutd.ap().rearrange("(k p) n -> p k n", p=128))
        sb_bout = singles.tile([1, 256], bf16)
        nc.scalar.dma_start(out=sb_bout, in_=boutd.ap())
        sb_ones = singles.tile([1, 128], bf16)
        nc.vector.memset(sb_ones, 1.0)
        ident = singles.tile([128, 128], bf16)
        make_identity(nc, ident[:])

        # clip-hi per (l, xy): xy=0 -> w-4, xy=1 -> h-4
        wh4 = singles.tile([128, NL, 2], f32)
        for l in range(NL):
            nc.vector.memset(wh4[:, l, 0:1], float(W_L[l] - WIN))
            nc.vector.memset(wh4[:, l, 1:2], float(H_L[l] - WIN))
        wrow = singles.tile([128, NL], f32)
        for l in range(NL):
            nc.vector.memset(wrow[:, l:l + 1], float(W_L[l]))
        jw = singles.tile([128, NL, WIN], f32)
        for l in range(NL):
            for j in range(WIN):
                nc.vector.memset(jw[:, l, j:j + 1], float(j * W_L[l]))
        jneg = singles.tile([128, WIN], f32)
        for j in range(WIN):
            nc.vector.memset(jneg[:, j:j + 1], float(-j))
        # per-qt level base (batch offset included): partition p of tile it
        # holds (q0+p)//Q * S + BASE_L[l].
        baselv = singles.tile([128, NQT, NL], f32)
        for it in range(NQT):
            q0 = it * 128
            for l in range(NL):
                nc.vector.memset(baselv[:, it, l:l + 1], float(BASE_L[l]))
                if q0 + 127 >= Q and q0 < Q:
                    nc.gpsimd.affine_select(
                        out=baselv[:, it, l:l + 1],
                        in_=baselv[:, it, l:l + 1],
                        pattern=[[0, 1]], base=Q - 1 - q0,
                        channel_multiplier=-1,
                        compare_op=mybir.AluOpType.is_ge,
                        fill=float(S + BASE_L[l]))
                elif q0 >= Q:
                    nc.vector.memset(baselv[:, it, l:l + 1],
                                     float(S + BASE_L[l]))

        # ---------------- one-shot geometry for ALL tiles ----------------
        # refp[q, t, (l,xy)] ; all geometry in [128, NQT, 8]
        geo = singles.tile([128, 4, NQT, 2 * NL], f32)
        refp = geo[:, 0]
        xsc = geo[:, 1]
        pxm = geo[:, 2]
        vb = geo[:, 3]
        nc.sync.dma_start(out=refp,
                          in_=refd.ap().rearrange("(t p) x -> p t x", p=128))
        MAGIC = float(1 << 23)
        # floor(refp) via magic-add; vb-MAGIC-1, clamp into [0, wh-4]
        nc.vector.tensor_scalar(vb, refp, 0.5, MAGIC, OP.subtract, OP.add)
        nc.vector.tensor_scalar(xsc, vb, MAGIC + 1.0, 0.0, OP.subtract, OP.max)
        nc.vector.tensor_tensor(
            xsc[:, :, :], xsc[:, :, :],
            ap_of(wh4, 0, [[1, 128], [0, NQT], [1, 2 * NL]]), op=OP.min)
        nc.vector.tensor_sub(pxm, refp, xsc)
        # P0 = ysc*w + xsc + base; idx = P0 + j*w
        p0t = singles.tile([128, NQT, NL], f32)
        nc.vector.tensor_mul(
            p0t[:, :, :],
            ap_of(geo, 1 * NQT * 8 + 1, [[1, 128], [8, NQT], [2, NL]]),  # y of xsc
            ap_of(wrow, 0, [[1, 128], [0, NQT], [1, NL]]))
        nc.vector.tensor_add(
            p0t[:, :, :], p0t[:, :, :],
            ap_of(geo, 1 * NQT * 8, [[1, 128], [8, NQT], [2, NL]]))       # x of xsc
        nc.vector.tensor_add(p0t[:, :, :], p0t[:, :, :], baselv[:, :, :])
        idxf = singles.tile([128, NQT, NL, WIN], f32)
        nc.vector.tensor_add(
            idxf[:, :, :, :],
            ap_of(p0t, 0, [[1, 128], [NL, NQT], [1, NL], [0, WIN]]),
            ap_of(jw, 0, [[1, 128], [0, NQT], [WIN, NL], [1, WIN]]))
        idxi = singles.tile([128, NQT, NL * WIN], i16)
        nc.vector.tensor_copy(
            idxi[:, :, :],
            idxf[:, :, :, :].rearrange("q t l j -> q t (l j)"))
        # one 4-dim write wraps ALL tiles' indices into dma_gather layout
        # (flat k = j*128 + q at idxd[t, l*512 + k]); read back per 16-
        # partition group (8 DMAs spread over both HWDGE engines) instead
        # of 40 tiny replication reads that serialized on Sync.
        nc.sync.dma_start(
            out=ap_of(idxd[:, :], 0, [[1, 128], [2048, NQT], [512, NL], [128, WIN]]),
            in_=idxi[:, :, :].rearrange("q t x -> q (t x)"))
        idxw = singles.tile([128, NQT, NL, 32], i16)
        for g in range(8):
            eng = nc.scalar if g % 2 else nc.sync
            eng.dma_start(
                out=idxw[16 * g:16 * (g + 1), :, :, :],
                in_=ap_of(idxd[:, :], 0, [[1, 16], [2048, NQT], [16, NL * 32]]))

        # ---------------- per-tile projections -> offs/elog ----------------
        offs_all = singles.tile([128, NQT, 256], f32)
        elog_all = singles.tile([128, NQT, 128], f32)
        # pad rows (600..639 -> partitions 88..127 of the last tile) never
        # get projection results; define them so downstream one-shots read
        # finite values (partition offsets must be 32-aligned; rows 64..87
        # are overwritten by the real copies afterwards).
        nc.vector.memset(offs_all[64:128, NQT - 1, :], 0.0)
        nc.vector.memset(elog_all[64:128, NQT - 1, :], 1.0)
        for it in range(NQT):
            q0 = it * 128
            qlen = min(QS - q0, 128)
            ql = slice(0, qlen)
            ps_off = psum_mm.tile([128, 256], f32, tag="ps_off")
            nc.tensor.matmul(ps_off[ql, :], lhsT=sb_qT[:, 0, q0:q0 + qlen],
                             rhs=sb_Woff[:, 0, :], start=True, stop=False)
            nc.tensor.matmul(ps_off[ql, :], lhsT=sb_qT[:, 1, q0:q0 + qlen],
                             rhs=sb_Woff[:, 1, :], start=False, stop=True)
            ps_log = psum_mm.tile([128, 128], f32, tag="ps_log")
            nc.tensor.matmul(ps_log[ql, :], lhsT=sb_qT[:, 0, q0:q0 + qlen],
                             rhs=sb_Wattn[:, 0, :], start=True, stop=False)
            nc.tensor.matmul(ps_log[ql, :], lhsT=sb_qT[:, 1, q0:q0 + qlen],
                             rhs=sb_Wattn[:, 1, :], start=False, stop=True)
            nc.scalar.copy(offs_all[ql, it, :], ps_off[ql, :])
            nc.scalar.activation(elog_all[ql, it, :], ps_log[ql, :], AF.Exp)

        # ---------------- one-shot softmax over (l,p) per h ----------------
        ssum = singles.tile([128, NQT, NH], f32)
        nc.vector.tensor_reduce(
            ssum[:, :, :],
            elog_all[:, :, :].rearrange("q t (h s) -> q t h s", h=NH),
            axis=mybir.AxisListType.X, op=OP.add)
        rinv = singles.tile([128, NQT, NH], f32)
        nc.vector.reciprocal(rinv[:, :, :].rearrange("q t h -> q (t h)"),
                             ssum[:, :, :].rearrange("q t h -> q (t h)"))
        # attnR[q, t, (l,h,p)] = elog[q, t, (h,l,p)] * rinv[q, t, h]
        attnR = singles.tile([128, NQT, 128], f32)
        for it in range(NQT):
            nc.vector.tensor_mul(
                attnR[:, it, :],
                ap_of(elog_all, it * 128, [[1, 128], [4, NL], [16, NH], [1, NP]]),
                ap_of(rinv, it * NH, [[1, 128], [0, NL], [1, NH], [0, NP]]))

        # ---------------- one-shot U and hats ----------------
        # U[q, t, l, xy, hp] = offs[q, t, (l,h,p,xy)] + pxm[q, t, (l,xy)]
        uu = singles.tile([128, NQT, NL, 2, 32], f32)
        for it in range(NQT):
            nc.gpsimd.tensor_add(
                uu[:, it, :, :, :],
                ap_of(offs_all, it * 256, [[1, 128], [64, NL], [1, 2], [2, 32]]),
                ap_of(geo, 2 * NQT * 8 + it * 8, [[1, 128], [2, NL], [1, 2], [0, 32]]))
        # hat[q, t, j, l, xy, hp] = relu(1 - |U - j|)
        hat = singles.tile([128, NQT, WIN, NL, 2, 32], f32)
        for it in range(NQT):
            for j in range(WIN):
                nc.scalar.activation(hat[:, it, j, :, :, :],
                                     uu[:, it, :, :, :], AF.Abs,
                                     bias=jneg[:, j:j + 1])
        nc.scalar.activation(
            hat[:, :, :, :, :, :].rearrange("q t j l x s -> q (t j l x s)"),
            hat[:, :, :, :, :, :].rearrange("q t j l x s -> q (t j l x s)"),
            AF.Relu, bias=1.0, scale=-1.0)
        # AFY[q, t, (l,h,p,i)] = attnR[q,t,(l,h,p)] * haty[q,t,(i,l,hp)]
        afy = singles.tile([128, NQT, NL, 8, NP, WIN], f32)
        for it in range(NQT):
            nc.vector.tensor_mul(
                afy[:, it, :, :, :, :],
                ap_of(hat, it * 1024 + 32, [[1, 128], [64, NL], [1, 32], [256, WIN]]),
                ap_of(attnR, it * 128, [[1, 128], [32, NL], [1, 32], [0, WIN]]))

        # ---------------- main loop: gather + stencil + reduce ----------------
        for it in range(NQT):
            q0 = it * 128
            qlen = QT_SIZES[it]
            ql = slice(0, qlen)
            res4 = work.tile([128, NL, 256], bf16, tag="res4")
            for l in range(NL):
                win = winp.tile([128, WIN, ELEM], bf16, tag="win")
                nc.gpsimd.dma_gather(
                    out_ap=win[:, :, :],
                    in_ap=ap_of(memd.ap(), 0, [[C, MEMROWS - (WIN - 1)], [1, ELEM]]),
                    idxs_ap=idxw[:, it, l, :],
                    num_idxs=512, num_idxs_reg=512,
                    elem_size=ELEM, elem_step=C,
                    queue_num=l % 4)

                # prod[q, (h,i,j), p] = afy[q,t,(l,h,p,i)] * hatx[q,t,(j,l,hp)]
                prod = work.tile([128, 8 * WIN * WIN, NP], f32, tag="prod")
                for p in range(NP):
                    nc.gpsimd.tensor_mul(
                        ap_of(prod, p, [[1, qlen], [NP, 8 * WIN * WIN]]),
                        ap_of(afy, it * 512 + l * 128 + p * WIN,
                              [[1, qlen], [16, 8], [1, WIN], [0, WIN]]),
                        ap_of(hat, it * 1024 + l * 64 + p,
                              [[1, qlen], [4, 8], [0, WIN], [256, WIN]]))
                mmh = work.tile([128, 8 * WIN * WIN, 2], f32, tag="mmh")
                nc.vector.tensor_add(
                    mmh[ql, :, :],
                    ap_of(prod, 0, [[1, qlen], [NP, 8 * WIN * WIN], [1, 2]]),
                    ap_of(prod, 2, [[1, qlen], [NP, 8 * WIN * WIN], [1, 2]]))
                mm = work.tile([128, 8, WIN, WIN], f32, tag="mm")
                nc.vector.tensor_add(
                    mm[ql, :, :, :].rearrange("q h i j -> q (h i j)"),
                    ap_of(mmh, 0, [[1, qlen], [2, 8 * WIN * WIN]]),
                    ap_of(mmh, 1, [[1, qlen], [2, 8 * WIN * WIN]]))
                # ME16[q, (i,j,h)] = mm[q, (h,i,j)]  (bf16, head innermost)
                me16 = work.tile([128, WIN, WIN, 8], bf16, tag="me16")
                nc.vector.tensor_copy(
                    me16[ql, :, :, :],
                    ap_of(mm, 0, [[1, qlen], [4, WIN], [1, WIN], [16, 8]]))
                # winM[q, px, d, h] = win[q, px, (d,h)] * ME16[q, px, h]
                # inner run of 8 (head dim) unit-stride -> DVE 2x bf16
                winM = work.tile([128, 4096], bf16, tag="winM")
                nc.vector.tensor_mul(
                    ap_of(winM, 0, [[1, qlen], [256, 16], [8, 32], [1, 8]]),
                    ap_of(win, 0, [[1, qlen], [256, 16], [8, 32], [1, 8]]),
                    ap_of(me16, 0, [[1, qlen], [8, 16], [0, 32], [1, 8]]))
                # pixel-sum: 4-stage pairwise tree, all bf16 2x adds
                t1 = work.tile([128, 2048], bf16, tag="t1")
                nc.vector.tensor_add(t1[ql, :], winM[ql, 0:2048],
                                     winM[ql, 2048:4096])
                t2 = work.tile([128, 1024], bf16, tag="t2")
                nc.vector.tensor_add(t2[ql, :], t1[ql, 0:1024], t1[ql, 1024:2048])
                t3 = work.tile([128, 512], bf16, tag="t3")
                nc.vector.tensor_add(t3[ql, :], t2[ql, 0:512], t2[ql, 512:1024])
                nc.vector.tensor_add(res4[ql, l, :], t3[ql, 0:256], t3[ql, 256:512])

            # sum over levels (tree, bf16)
            nc.vector.tensor_add(res4[ql, 0:2, :], res4[ql, 0:2, :], res4[ql, 2:4, :])
            res = work.tile([128, 256], bf16, tag="res")
            nc.vector.tensor_add(res[ql, :], res4[ql, 0, :], res4[ql, 1, :])

            # --- output projection: out = res @ Wout + bout (bf16 PE)
            resT = work.tile([128, 2, 128], bf16, tag="resT")
            for hh in range(2):
                ps_t = psum_tr.tile([128, 128], bf16, tag="ps_t")
                nc.tensor.transpose(ps_t[:, ql], res[ql, 128 * hh:128 * (hh + 1)],
                                    ident[ql, ql])
                nc.scalar.copy(resT[:, hh, ql], ps_t[:, ql])
            ps_out = psum_o.tile([128, 256], f32, tag="ps_out")
            nc.tensor.matmul(ps_out[ql, :], lhsT=resT[:, 0, ql],
                             rhs=sb_Wout[:, 0, :], start=True, stop=False)
            nc.tensor.matmul(ps_out[ql, :], lhsT=resT[:, 1, ql],
                             rhs=sb_Wout[:, 1, :], start=False, stop=False)
            nc.tensor.matmul(ps_out[ql, :], lhsT=sb_ones[0:1, ql],
                             rhs=sb_bout[0:1, :], start=False, stop=True)
            outt = work.tile([128, 256], f32, tag="outt")
            nc.scalar.copy(outt[ql, :], ps_out[ql, :])
            qlen_out = min(QS - q0, 128)
            eng = nc.scalar if it % 2 else nc.sync
            eng.dma_start(out=outd.ap()[q0:q0 + qlen_out, :],
                          in_=outt[0:qlen_out, :])

    nc.compile()
    return nc


_NC_CACHE = {}
LAST_RESULTS = None


def _get_nc():
    if "nc" not in _NC_CACHE:
        _NC_CACHE["nc"] = _build_program()
    return _NC_CACHE["nc"]


def host_prep(query, memory, ref_points, W_off, b_off, W_attn, b_attn,
              W_out, b_out):
    """Build the 8 per-core input maps (pure layout/dtype transforms)."""
    import ml_dtypes
    bf16 = ml_dtypes.bfloat16

    query = np.ascontiguousarray(query, dtype=np.float32)
    memory = np.ascontiguousarray(memory, dtype=np.float32)
    ref = np.asarray(ref_points, dtype=np.float32)
    W_off = np.asarray(W_off, dtype=np.float32)
    b_off = np.asarray(b_off, dtype=np.float32)
    W_attn = np.asarray(W_attn, dtype=np.float32)
    b_attn = np.asarray(b_attn, dtype=np.float32)
    assert np.all(b_off == 0.0) and np.all(b_attn == 0.0), \
        "nonzero offset/attn biases not folded on device"
    # W_off cols (h,l,p,xy) -> (l,h,p,xy)
    Woff_r = np.ascontiguousarray(
        W_off.reshape(C, NH, NL, NP, 2).transpose(0, 2, 1, 3, 4)
        .reshape(C, 256)).astype(bf16)
    Wattn_r = np.ascontiguousarray(W_attn).astype(bf16)
    # W_out rows permuted to the (d, h) channel order the window path uses
    Wout_r = np.ascontiguousarray(
        np.asarray(W_out, np.float32).reshape(NH, HD, C)
        .transpose(1, 0, 2).reshape(C, C)).astype(bf16)
    bout = np.ascontiguousarray(
        np.asarray(b_out, dtype=np.float32).reshape(1, C)).astype(bf16)

    wh = np.array([[w, h] for h, w in SPATIAL], dtype=np.float32)
    in_maps = []
    for c in range(NCORES):
        bs = slice(BPC * c, BPC * (c + 1))
        qT = np.ascontiguousarray(
            query[bs].reshape(QS, C).T).astype(bf16)          # [256, 600]
        # memory channels (h,d) -> (d,h), bf16
        mem = np.ascontiguousarray(
            memory[bs].reshape(MEMROWS, NH, HD).transpose(0, 2, 1)
            .reshape(MEMROWS, C)).astype(bf16)
        refc = ref[bs].reshape(QS, NL, 2)
        refpix = refc * wh[None, :, :] - 0.5                   # [600, l, xy]
        refpix = refpix.reshape(QS, 2 * NL).astype(np.float32)
        refpad = np.full((QSP, 2 * NL), 5.0, np.float32)       # safe interior
        refpad[:QS] = refpix
        in_maps.append(dict(mem=mem, qT=qT, refpix=np.ascontiguousarray(refpad),
                            Woff=Woff_r, Wattn=Wattn_r, Wout=Wout_r, bout=bout))
    return in_maps


def kernel(**inputs):
    global LAST_RESULTS
    from concourse.bass_utils import run_bass_kernel_spmd

    nc = _get_nc()
    in_maps = host_prep(**inputs)
    trace = bool(int(os.environ.get("KERNEL_TRACE", "0")))
    res = run_bass_kernel_spmd(nc, in_maps, core_ids=list(range(NCORES)),
                               trace=trace)
    LAST_RESULTS = res
    out = np.empty((B, Q, C), dtype=np.float32)
    for c in range(NCORES):
        out[BPC * c:BPC * (c + 1)] = res.results[c]["out"].reshape(BPC, Q, C)
    return out


# revision 13
# speedup vs baseline: 1.4181x; 1.2815x over previous
"""Trainium2 Bass kernel for DEIM multi-scale deformable attention.

Strategy:
  - Data-parallel over batch: 16 batches -> 8 cores, 2 batches/core.
  - Per (b,q,level): ONE 4x4-pixel x 256-channel bf16 window gather
    (4 descriptors of 2KB) instead of per-corner gathers; exact bilinear
    hat weights vs the window grid reproduce grid_sample(zeros) exactly.
  - M[h, iy, jx] = sum_p attn[h,p]*haty[h,p,iy]*hatx[h,p,jx] folds softmax
    attention + bilinear interp into a 16-pixel stencil per head.
  - Channels are HOST-permuted to (d, h) order (head index innermost) so the
    window multiply's broadcast of M over the 32 head-dims has a unit-stride
    innermost run of 8 -> DVE 2x_1p bf16 mode. The whole window path is bf16.
  - Pixel-sum as a 4-stage pairwise tree of bf16 2x adds.
  - Window placement depends only on ref_points, so the HOST precomputes the
    gather indices directly in dma_gather's wrapped+replicated idx layout
    (one contiguous load) and the window remainders pxm; the device keeps all
    value compute (projections, softmax, hats, stencil, sampling, out proj).
  - Engine split: DVE = window multiply + tree + stencil sums; POOL =
    gathers (4 SWDGE queues) + stencil products; ACT = hats + PSUM
    evacuations; PE = projections/transposes.
"""

import os
from contextlib import ExitStack

import numpy as np

# ---------------------------------------------------------------------------
# Problem constants (hardcoded per harness contract)
# ---------------------------------------------------------------------------
B, Q, C, NH, NP, NL = 16, 300, 256, 8, 4, 4
HD = C // NH
SPATIAL = ((80, 80), (40, 40), (20, 20), (30, 70))  # (h, w) per level
S = sum(h * w for h, w in SPATIAL)  # 10500
BASE_L = [0, 6400, 8000, 8400]
H_L = [h for h, w in SPATIAL]
W_L = [w for h, w in SPATIAL]

NCORES = 8
BPC = B // NCORES          # batches per core
QS = BPC * Q               # query slots per core (600)
QSP = 640                  # padded query slots (5 x 128)
NQT = 5
MEMROWS = BPC * S          # 21000 pixel rows per core
WIN = 4                    # window size (pixels per axis)
ELEM = WIN * C             # gather element: 4 pixels x 256 ch
IDXC = NL * 32             # idx cols per tile in wrapped layout


def _build_program():
    import concourse.bacc as bacc
    import concourse.bass as bass
    import concourse.tile as tile
    from concourse import mybir
    from concourse.masks import make_identity

    f32 = mybir.dt.float32
    bf16 = mybir.dt.bfloat16
    i16 = mybir.dt.int16

    nc = bacc.Bacc("TRN2", target_bir_lowering=False, debug=False,
                   num_devices=NCORES, num_swdge_queues=4)

    AF = mybir.ActivationFunctionType
    OP = mybir.AluOpType

    def ap_of(t, off, pairs):
        a = t[:] if hasattr(t, "__getitem__") else t
        pairs = [list(p) for p in pairs]
        if a.space == bass.MemorySpace.SBUF:
            pairs[0][0] *= a.ap[0][0]
        return bass.AP(tensor=a.tensor, offset=a.offset + off, ap=pairs)

    # ------------------------------------------------------------------
    # DRAM I/O  (mem/qT/weights bf16; channel order (d, h))
    # ------------------------------------------------------------------
    memd = nc.dram_tensor("mem", [MEMROWS, C], bf16, kind="ExternalInput")
    qTd = nc.dram_tensor("qT", [C, QS], bf16, kind="ExternalInput")
    pxmd = nc.dram_tensor("pxm", [QSP, 2 * NL], f32, kind="ExternalInput")
    idxwd = nc.dram_tensor("idxw", [128, NQT * IDXC], i16, kind="ExternalInput")
    woffd = nc.dram_tensor("Woff", [C, 256], bf16, kind="ExternalInput")
    wattnd = nc.dram_tensor("Wattn", [C, NH * NL * NP], bf16, kind="ExternalInput")
    woutd = nc.dram_tensor("Wout", [C, C], bf16, kind="ExternalInput")
    boutd = nc.dram_tensor("bout", [1, C], bf16, kind="ExternalInput")
    outd = nc.dram_tensor("out", [QS, C], f32, kind="ExternalOutput")

    with tile.TileContext(nc) as tc, ExitStack() as ctx:
        singles = ctx.enter_context(tc.tile_pool(name="singles", bufs=1))
        psum_mm = ctx.enter_context(tc.tile_pool(name="psum_mm", bufs=2, space="PSUM"))
        psum_tr = ctx.enter_context(tc.tile_pool(name="psum_tr", bufs=2, space="PSUM"))
        psum_o = ctx.enter_context(tc.tile_pool(name="psum_o", bufs=2, space="PSUM"))
        work = ctx.enter_context(tc.tile_pool(name="work", bufs=3))
        winp = ctx.enter_context(tc.tile_pool(name="winp", bufs=4))

        # ---------------- one-time loads ----------------
        idxw = singles.tile([128, NQT, IDXC], i16)
        nc.scalar.dma_start(out=idxw[:, :, :].rearrange("p t c -> p (t c)"),
                            in_=idxwd.ap())
        sb_qT = singles.tile([128, 2, QS], bf16)
        nc.sync.dma_start(out=sb_qT, in_=qTd.ap().rearrange("(k p) q -> p k q", p=128))
        sb_Woff = singles.tile([128, 2, 256], bf16)
        nc.scalar.dma_start(out=sb_Woff, in_=woffd.ap().rearrange("(k p) n -> p k n", p=128))
        sb_Wattn = singles.tile([128, 2, 128], bf16)
        nc.scalar.dma_start(out=sb_Wattn, in_=wattnd.ap().rearrange("(k p) n -> p k n", p=128))
        sb_Wout = singles.tile([128, 2, 256], bf16)
        nc.scalar.dma_start(out=sb_Wout, in_=woutd.ap().rearrange("(k p) n -> p k n", p=128))
        sb_bout = singles.tile([1, 256], bf16)
        nc.scalar.dma_start(out=sb_bout, in_=boutd.ap())
        pxm = singles.tile([128, NQT, 2 * NL], f32)
        nc.sync.dma_start(out=pxm,
                          in_=pxmd.ap().rearrange("(t p) x -> p t x", p=128))
        sb_ones = singles.tile([1, 128], bf16)
        nc.vector.memset(sb_ones, 1.0)
        ident = singles.tile([128, 128], bf16)
        make_identity(nc, ident[:])
        jneg = singles.tile([128, WIN], f32)
        for j in range(WIN):
            nc.vector.memset(jneg[:, j:j + 1], float(-j))

        # ---------------- per-tile projections -> offs/elog ----------------
        offs_all = singles.tile([128, NQT, 256], f32)
        elog_all = singles.tile([128, NQT, 128], f32)
        # pad rows (600..639 -> partitions 88..127 of the last tile) never get
        # projection results; define them for the one-shot softmax/hat reads
        # (rows 64..87 are overwritten by the real copies afterwards).
        nc.vector.memset(offs_all[64:128, NQT - 1, :], 0.0)
        nc.vector.memset(elog_all[64:128, NQT - 1, :], 1.0)
        for it in range(NQT):
            q0 = it * 128
            qlen = min(QS - q0, 128)
            ql = slice(0, qlen)
            ps_off = psum_mm.tile([128, 256], f32, tag="ps_off")
            nc.tensor.matmul(ps_off[ql, :], lhsT=sb_qT[:, 0, q0:q0 + qlen],
                             rhs=sb_Woff[:, 0, :], start=True, stop=False)
            nc.tensor.matmul(ps_off[ql, :], lhsT=sb_qT[:, 1, q0:q0 + qlen],
                             rhs=sb_Woff[:, 1, :], start=False, stop=True)
            ps_log = psum_mm.tile([128, 128], f32, tag="ps_log")
            nc.tensor.matmul(ps_log[ql, :], lhsT=sb_qT[:, 0, q0:q0 + qlen],
                             rhs=sb_Wattn[:, 0, :], start=True, stop=False)
            nc.tensor.matmul(ps_log[ql, :], lhsT=sb_qT[:, 1, q0:q0 + qlen],
                             rhs=sb_Wattn[:, 1, :], start=False, stop=True)
            nc.scalar.copy(offs_all[ql, it, :], ps_off[ql, :])
            nc.scalar.activation(elog_all[ql, it, :], ps_log[ql, :], AF.Exp)

        # ---------------- one-shot softmax over (l,p) per h ----------------
        ssum = singles.tile([128, NQT, NH], f32)
        nc.vector.tensor_reduce(
            ssum[:, :, :],
            elog_all[:, :, :].rearrange("q t (h s) -> q t h s", h=NH),
            axis=mybir.AxisListType.X, op=OP.add)
        rinv = singles.tile([128, NQT, NH], f32)
        nc.vector.reciprocal(rinv[:, :, :].rearrange("q t h -> q (t h)"),
                             ssum[:, :, :].rearrange("q t h -> q (t h)"))
        attnR = singles.tile([128, NQT, 128], f32)
        for it in range(NQT):
            nc.vector.tensor_mul(
                attnR[:, it, :],
                ap_of(elog_all, it * 128, [[1, 128], [4, NL], [16, NH], [1, NP]]),
                ap_of(rinv, it * NH, [[1, 128], [0, NL], [1, NH], [0, NP]]))

        # ---------------- one-shot U and hats ----------------
        uu = singles.tile([128, NQT, NL, 2, 32], f32)
        for it in range(NQT):
            nc.gpsimd.tensor_add(
                uu[:, it, :, :, :],
                ap_of(offs_all, it * 256, [[1, 128], [64, NL], [1, 2], [2, 32]]),
                ap_of(pxm, it * 8, [[1, 128], [2, NL], [1, 2], [0, 32]]))
        hat = singles.tile([128, NQT, WIN, NL, 2, 32], f32)
        for it in range(NQT):
            for j in range(WIN):
                nc.scalar.activation(hat[:, it, j, :, :, :],
                                     uu[:, it, :, :, :], AF.Abs,
                                     bias=jneg[:, j:j + 1])
        nc.scalar.activation(
            hat[:, :, :, :, :, :].rearrange("q t j l x s -> q (t j l x s)"),
            hat[:, :, :, :, :, :].rearrange("q t j l x s -> q (t j l x s)"),
            AF.Relu, bias=1.0, scale=-1.0)
        afy = singles.tile([128, NQT, NL, 8, NP, WIN], f32)
        for it in range(NQT):
            nc.vector.tensor_mul(
                afy[:, it, :, :, :, :],
                ap_of(hat, it * 1024 + 32, [[1, 128], [64, NL], [1, 32], [256, WIN]]),
                ap_of(attnR, it * 128, [[1, 128], [32, NL], [1, 32], [0, WIN]]))

        # ---------------- main loop: gather + stencil + reduce ----------------
        for it in range(NQT):
            q0 = it * 128
            ql = slice(0, 128)
            qlen = 128
            res4 = work.tile([128, NL, 256], bf16, tag="res4")
            for l in range(NL):
                win = winp.tile([128, WIN, ELEM], bf16, tag="win")
                nc.gpsimd.dma_gather(
                    out_ap=win[:, :, :],
                    in_ap=ap_of(memd.ap(), 0, [[C, MEMROWS - (WIN - 1)], [1, ELEM]]),
                    idxs_ap=idxw[:, it, l * 32:(l + 1) * 32],
                    num_idxs=512, num_idxs_reg=512,
                    elem_size=ELEM, elem_step=C,
                    queue_num=l % 4)

                # prod[q, (h,i,j), p] = afy[q,t,(l,h,p,i)] * hatx[q,t,(j,l,hp)]
                prod = work.tile([128, 8 * WIN * WIN, NP], f32, tag="prod")
                for p in range(NP):
                    nc.gpsimd.tensor_mul(
                        ap_of(prod, p, [[1, qlen], [NP, 8 * WIN * WIN]]),
                        ap_of(afy, it * 512 + l * 128 + p * WIN,
                              [[1, qlen], [16, 8], [1, WIN], [0, WIN]]),
                        ap_of(hat, it * 1024 + l * 64 + p,
                              [[1, qlen], [4, 8], [0, WIN], [256, WIN]]))
                mmh = work.tile([128, 8 * WIN * WIN, 2], f32, tag="mmh")
                nc.vector.tensor_add(
                    mmh[ql, :, :],
                    ap_of(prod, 0, [[1, qlen], [NP, 8 * WIN * WIN], [1, 2]]),
                    ap_of(prod, 2, [[1, qlen], [NP, 8 * WIN * WIN], [1, 2]]))
                mm = work.tile([128, 8, WIN, WIN], f32, tag="mm")
                nc.vector.tensor_add(
                    mm[ql, :, :, :].rearrange("q h i j -> q (h i j)"),
                    ap_of(mmh, 0, [[1, qlen], [2, 8 * WIN * WIN]]),
                    ap_of(mmh, 1, [[1, qlen], [2, 8 * WIN * WIN]]))
                me16 = work.tile([128, WIN, WIN, 8], bf16, tag="me16")
                nc.vector.tensor_copy(
                    me16[ql, :, :, :],
                    ap_of(mm, 0, [[1, qlen], [4, WIN], [1, WIN], [16, 8]]))
                # winM[q, px, d, h] = win[q, px, (d,h)] * ME16[q, px, h]
                winM = work.tile([128, 4096], bf16, tag="winM")
                nc.vector.tensor_mul(
                    ap_of(winM, 0, [[1, qlen], [256, 16], [8, 32], [1, 8]]),
                    ap_of(win, 0, [[1, qlen], [256, 16], [8, 32], [1, 8]]),
                    ap_of(me16, 0, [[1, qlen], [8, 16], [0, 32], [1, 8]]))
                t1 = work.tile([128, 2048], bf16, tag="t1")
                nc.vector.tensor_add(t1[ql, :], winM[ql, 0:2048],
                                     winM[ql, 2048:4096])
                t2 = work.tile([128, 1024], bf16, tag="t2")
                nc.vector.tensor_add(t2[ql, :], t1[ql, 0:1024], t1[ql, 1024:2048])
                t3 = work.tile([128, 512], bf16, tag="t3")
                nc.vector.tensor_add(t3[ql, :], t2[ql, 0:512], t2[ql, 512:1024])
                nc.vector.tensor_add(res4[ql, l, :], t3[ql, 0:256], t3[ql, 256:512])

            nc.vector.tensor_add(res4[ql, 0:2, :], res4[ql, 0:2, :], res4[ql, 2:4, :])
            res = work.tile([128, 256], bf16, tag="res")
            nc.vector.tensor_add(res[ql, :], res4[ql, 0, :], res4[ql, 1, :])

            # --- output projection: out = res @ Wout + bout (bf16 PE)
            resT = work.tile([128, 2, 128], bf16, tag="resT")
            for hh in range(2):
                ps_t = psum_tr.tile([128, 128], bf16, tag="ps_t")
                nc.tensor.transpose(ps_t[:, ql], res[ql, 128 * hh:128 * (hh + 1)],
                                    ident[ql, ql])
                nc.scalar.copy(resT[:, hh, ql], ps_t[:, ql])
            ps_out = psum_o.tile([128, 256], f32, tag="ps_out")
            nc.tensor.matmul(ps_out[ql, :], lhsT=resT[:, 0, ql],
                             rhs=sb_Wout[:, 0, :], start=True, stop=False)
            nc.tensor.matmul(ps_out[ql, :], lhsT=resT[:, 1, ql],
                             rhs=sb_Wout[:, 1, :], start=False, stop=False)
            nc.tensor.matmul(ps_out[ql, :], lhsT=sb_ones[0:1, ql],
                             rhs=sb_bout[0:1, :], start=False, stop=True)
            outt = work.tile([128, 256], f32, tag="outt")
            nc.scalar.copy(outt[ql, :], ps_out[ql, :])
            qlen_out = min(QS - q0, 128)
            eng = nc.scalar if it % 2 else nc.sync
            eng.dma_start(out=outd.ap()[q0:q0 + qlen_out, :],
                          in_=outt[0:qlen_out, :])

    nc.compile()
    return nc


_NC_CACHE = {}
LAST_RESULTS = None


def _get_nc():
    if "nc" not in _NC_CACHE:
        _NC_CACHE["nc"] = _build_program()
    return _NC_CACHE["nc"]


def host_prep(query, memory, ref_points, W_off, b_off, W_attn, b_attn,
              W_out, b_out):
    """Build the 8 per-core input maps (layout/dtype transforms + gather-
    index addressing; all value compute stays on device)."""
    import ml_dtypes
    bf16 = ml_dtypes.bfloat16

    query = np.ascontiguousarray(query, dtype=np.float32)
    memory = np.ascontiguousarray(memory, dtype=np.float32)
    ref = np.asarray(ref_points, dtype=np.float32)
    W_off = np.asarray(W_off, dtype=np.float32)
    b_off = np.asarray(b_off, dtype=np.float32)
    W_attn = np.asarray(W_attn, dtype=np.float32)
    b_attn = np.asarray(b_attn, dtype=np.float32)
    assert np.all(b_off == 0.0) and np.all(b_attn == 0.0), \
        "nonzero offset/attn biases not folded on device"
    Woff_r = np.ascontiguousarray(
        W_off.reshape(C, NH, NL, NP, 2).transpose(0, 2, 1, 3, 4)
        .reshape(C, 256)).astype(bf16)
    Wattn_r = np.ascontiguousarray(W_attn).astype(bf16)
    Wout_r = np.ascontiguousarray(
        np.asarray(W_out, np.float32).reshape(NH, HD, C)
        .transpose(1, 0, 2).reshape(C, C)).astype(bf16)
    bout = np.ascontiguousarray(
        np.asarray(b_out, dtype=np.float32).reshape(1, C)).astype(bf16)

    wh = np.array([[w, h] for h, w in SPATIAL], dtype=np.float32)
    wh4 = np.array([[w - WIN, h - WIN] for h, w in SPATIAL], np.float32)
    wvec = np.array(W_L, np.float32)
    base = np.array(BASE_L, np.float32)

    in_maps = []
    for c in range(NCORES):
        bs = slice(BPC * c, BPC * (c + 1))
        qT = np.ascontiguousarray(
            query[bs].reshape(QS, C).T).astype(bf16)
        mem = np.ascontiguousarray(
            memory[bs].reshape(MEMROWS, NH, HD).transpose(0, 2, 1)
            .reshape(MEMROWS, C)).astype(bf16)
        refc = ref[bs].reshape(QS, NL, 2)
        refpix = refc * wh[None, :, :] - 0.5                   # [600, l, xy]
        refpad = np.full((QSP, NL, 2), 5.0, np.float32)
        refpad[:QS] = refpix
        # window start (clip into grid) + remainder; gather row indices
        xsc = np.clip(np.floor(refpad) - 1.0, 0.0, wh4[None, :, :])
        pxm = (refpad - xsc).reshape(QSP, NL * 2).astype(np.float32)
        batch = (np.arange(QSP) // Q).clip(max=BPC - 1).astype(np.float32)
        p0 = (xsc[:, :, 1] * wvec[None, :] + xsc[:, :, 0]
              + base[None, :] + batch[:, None] * S)            # [640, l]
        idx = (p0[:, :, None] + np.arange(WIN)[None, None, :]
               * wvec[None, :, None]).astype(np.int64)         # [640, l, j]
        # dma_gather wrapped layout: flat k = j*128 + q per (tile, level),
        # idx col c = j*8 + q//16, partition p = q%16, replicated x8 groups
        idxw = np.empty((128, NQT, NL, 32), np.int16)
        q_of = (np.arange(32) % 8) * 16                        # c -> q base
        j_of = np.arange(32) // 8                              # c -> j
        for t in range(NQT):
            for pp in range(16):
                qq = t * 128 + q_of + pp                       # [32] query slot
                idxw[pp, t, :, :] = idx[qq, :, j_of].transpose(1, 0)
        idxw = np.ascontiguousarray(
            np.broadcast_to(idxw[:16], (8, 16, NQT, NL, 32))
            .reshape(128, NQT * NL * 32))
        in_maps.append(dict(mem=mem, qT=qT, pxm=pxm,
                            idxw=idxw, Woff=Woff_r,
                            Wattn=Wattn_r, Wout=Wout_r, bout=bout))
    return in_maps


def kernel(**inputs):
    global LAST_RESULTS
    from concourse.bass_utils import run_bass_kernel_spmd

    nc = _get_nc()
    in_maps = host_prep(**inputs)
    trace = bool(int(os.environ.get("KERNEL_TRACE", "0")))
    res = run_bass_kernel_spmd(nc, in_maps, core_ids=list(range(NCORES)),
                               trace=trace)
    LAST_RESULTS = res
    out = np.empty((B, Q, C), dtype=np.float32)
    for c in range(NCORES):
        out[BPC * c:BPC * (c + 1)] = res.results[c]["out"].reshape(BPC, Q, C)
    return out


# revision 15
# speedup vs baseline: 3.7671x; 2.6565x over previous
"""Trainium2 Bass kernel for DEIM multi-scale deformable attention.

Strategy:
  - Data-parallel over batch: 16 batches -> 8 cores, 2 batches/core.
  - Per (b,q,level): ONE 4x4-pixel x 256-channel bf16 window gather
    (4 descriptors of 2KB); exact bilinear hat weights vs the window grid
    reproduce grid_sample(zeros) exactly.
  - M[h, iy, jx] = sum_p attn[h,p]*haty[h,p,iy]*hatx[h,p,jx] folds softmax
    attention + bilinear interp into a 16-pixel stencil per head.
  - Channels HOST-permuted to (d, h) order (head innermost) so the window
    multiply's M-broadcast has unit-stride runs of 8 -> DVE 2x bf16 mode.
  - Pixel-sum as a 4-stage pairwise tree of bf16 2x adds.
  - Window placement depends only on ref_points -> HOST precomputes gather
    indices in dma_gather's wrapped layout (one contiguous load) + window
    remainders pxm. All value compute stays on device.
  - Fully per-tile pipeline (proj -> softmax -> hats -> 4 levels) so tile
    t+1's prep overlaps tile t's windows. POOL runs ONLY dma_gathers (no
    ucode library thrash); DVE does stencil+window math; ACT does hats,
    casts, PSUM evacuation; PE does projections.
"""

import os
from contextlib import ExitStack

import numpy as np

B, Q, C, NH, NP, NL = 16, 300, 256, 8, 4, 4
HD = C // NH
SPATIAL = ((80, 80), (40, 40), (20, 20), (30, 70))
S = sum(h * w for h, w in SPATIAL)
BASE_L = [0, 6400, 8000, 8400]
H_L = [h for h, w in SPATIAL]
W_L = [w for h, w in SPATIAL]

NCORES = 8
BPC = B // NCORES
QS = BPC * Q               # 600
QSP = 640
NQT = 5
MEMROWS = BPC * S          # 21000
WIN = 4
ELEM = WIN * C
IDXC = NL * 32


def _build_program():
    import concourse.bacc as bacc
    import concourse.bass as bass
    import concourse.tile as tile
    from concourse import mybir
    from concourse.masks import make_identity

    f32 = mybir.dt.float32
    bf16 = mybir.dt.bfloat16
    i16 = mybir.dt.int16

    nc = bacc.Bacc("TRN2", target_bir_lowering=False, debug=False,
                   num_devices=NCORES, num_swdge_queues=4)

    AF = mybir.ActivationFunctionType
    OP = mybir.AluOpType

    def ap_of(t, off, pairs):
        a = t[:] if hasattr(t, "__getitem__") else t
        pairs = [list(p) for p in pairs]
        if a.space == bass.MemorySpace.SBUF:
            pairs[0][0] *= a.ap[0][0]
        return bass.AP(tensor=a.tensor, offset=a.offset + off, ap=pairs)

    memd = nc.dram_tensor("mem", [MEMROWS, C], bf16, kind="ExternalInput")
    qTd = nc.dram_tensor("qT", [C, QS], bf16, kind="ExternalInput")
    pxmd = nc.dram_tensor("pxm", [QSP, 2 * NL], f32, kind="ExternalInput")
    idxwd = nc.dram_tensor("idxw", [128, NQT * IDXC], i16, kind="ExternalInput")
    woffd = nc.dram_tensor("Woff", [C, 256], bf16, kind="ExternalInput")
    wattnd = nc.dram_tensor("Wattn", [C, NH * NL * NP], bf16, kind="ExternalInput")
    woutd = nc.dram_tensor("Wout", [C, C], bf16, kind="ExternalInput")
    boutd = nc.dram_tensor("bout", [1, C], bf16, kind="ExternalInput")
    outd = nc.dram_tensor("out", [QS, C], f32, kind="ExternalOutput")

    with tile.TileContext(nc) as tc, ExitStack() as ctx:
        singles = ctx.enter_context(tc.tile_pool(name="singles", bufs=1))
        psum_mm = ctx.enter_context(tc.tile_pool(name="psum_mm", bufs=2, space="PSUM"))
        psum_tr = ctx.enter_context(tc.tile_pool(name="psum_tr", bufs=2, space="PSUM"))
        psum_o = ctx.enter_context(tc.tile_pool(name="psum_o", bufs=2, space="PSUM"))
        work = ctx.enter_context(tc.tile_pool(name="work", bufs=3))
        winp = ctx.enter_context(tc.tile_pool(name="winp", bufs=4))

        # ---------------- one-time loads ----------------
        idxw = singles.tile([128, NQT, IDXC], i16)
        nc.scalar.dma_start(out=idxw[:, :, :].rearrange("p t c -> p (t c)"),
                            in_=idxwd.ap())
        sb_qT = singles.tile([128, 2, QS], bf16)
        nc.sync.dma_start(out=sb_qT, in_=qTd.ap().rearrange("(k p) q -> p k q", p=128))
        sb_Woff = singles.tile([128, 2, 256], bf16)
        nc.scalar.dma_start(out=sb_Woff, in_=woffd.ap().rearrange("(k p) n -> p k n", p=128))
        sb_Wattn = singles.tile([128, 2, 128], bf16)
        nc.scalar.dma_start(out=sb_Wattn, in_=wattnd.ap().rearrange("(k p) n -> p k n", p=128))
        sb_Wout = singles.tile([128, 2, 256], bf16)
        nc.scalar.dma_start(out=sb_Wout, in_=woutd.ap().rearrange("(k p) n -> p k n", p=128))
        sb_bout = singles.tile([1, 256], bf16)
        nc.scalar.dma_start(out=sb_bout, in_=boutd.ap())
        pxm = singles.tile([128, NQT, 2 * NL], f32)
        nc.sync.dma_start(out=pxm,
                          in_=pxmd.ap().rearrange("(t p) x -> p t x", p=128))
        sb_ones = singles.tile([1, 128], bf16)
        nc.vector.memset(sb_ones, 1.0)
        ident = singles.tile([128, 128], bf16)
        make_identity(nc, ident[:])
        jneg = singles.tile([128, WIN], f32)
        for j in range(WIN):
            nc.vector.memset(jneg[:, j:j + 1], float(-j))

        # ---------------- per-tile fused pipeline ----------------
        for it in range(NQT):
            q0 = it * 128
            qlen = min(QS - q0, 128)
            ql = slice(0, qlen)
            qf = slice(0, 128)

            # projections (PE) -> offs / softmax (pad rows untouched: they
            # only ever produce values for discarded output rows)
            ps_off = psum_mm.tile([128, 256], f32, tag="ps_off")
            nc.tensor.matmul(ps_off[ql, :], lhsT=sb_qT[:, 0, q0:q0 + qlen],
                             rhs=sb_Woff[:, 0, :], start=True, stop=False)
            nc.tensor.matmul(ps_off[ql, :], lhsT=sb_qT[:, 1, q0:q0 + qlen],
                             rhs=sb_Woff[:, 1, :], start=False, stop=True)
            ps_log = psum_mm.tile([128, 128], f32, tag="ps_log")
            nc.tensor.matmul(ps_log[ql, :], lhsT=sb_qT[:, 0, q0:q0 + qlen],
                             rhs=sb_Wattn[:, 0, :], start=True, stop=False)
            nc.tensor.matmul(ps_log[ql, :], lhsT=sb_qT[:, 1, q0:q0 + qlen],
                             rhs=sb_Wattn[:, 1, :], start=False, stop=True)
            offs = work.tile([128, 256], f32, tag="offs")
            nc.scalar.copy(offs[ql, :], ps_off[ql, :])
            elog = work.tile([128, 128], f32, tag="elog")
            nc.scalar.activation(elog[ql, :], ps_log[ql, :], AF.Exp)

            ssum = work.tile([128, NH], f32, tag="ssum")
            nc.vector.tensor_reduce(
                ssum[ql, :],
                elog[ql, :].rearrange("q (h s) -> q h s", h=NH),
                axis=mybir.AxisListType.X, op=OP.add)
            rinv = work.tile([128, NH], f32, tag="rinv")
            nc.vector.reciprocal(rinv[ql, :], ssum[ql, :])
            attnR = work.tile([128, 128], f32, tag="attnR")
            nc.vector.tensor_mul(
                attnR[ql, :],
                ap_of(elog, 0, [[1, qlen], [4, NL], [16, NH], [1, NP]]),
                ap_of(rinv, 0, [[1, qlen], [0, NL], [1, NH], [0, NP]]))

            # U then hats (ACT) then attn-weighted y-hats (DVE)
            uu = work.tile([128, NL, 2, 32], f32, tag="uu")
            nc.vector.tensor_add(
                uu[ql, :, :, :],
                ap_of(offs, 0, [[1, qlen], [64, NL], [1, 2], [2, 32]]),
                ap_of(pxm, it * 8, [[1, qlen], [2, NL], [1, 2], [0, 32]]))
            hat = work.tile([128, WIN, NL, 2, 32], f32, tag="hat")
            for j in range(WIN):
                nc.scalar.activation(hat[ql, j, :, :, :],
                                     uu[ql, :, :, :], AF.Abs,
                                     bias=jneg[ql, j:j + 1])
            nc.scalar.activation(
                hat[ql, :, :, :, :].rearrange("q j l x s -> q (j l x s)"),
                hat[ql, :, :, :, :].rearrange("q j l x s -> q (j l x s)"),
                AF.Relu, bias=1.0, scale=-1.0)
            afy = work.tile([128, NL, 8, NP, WIN], f32, tag="afy")
            nc.vector.tensor_mul(
                afy[ql, :, :, :, :],
                ap_of(hat, 32, [[1, qlen], [64, NL], [1, 32], [256, WIN]]),
                ap_of(attnR, 0, [[1, qlen], [32, NL], [1, 32], [0, WIN]]))

            # ---- per level: gather (POOL) + stencil (DVE/ACT) + reduce ----
            res4 = work.tile([128, NL, 256], bf16, tag="res4")
            for l in range(NL):
                win = winp.tile([128, WIN, ELEM], bf16, tag="win")
                nc.gpsimd.dma_gather(
                    out_ap=win[:, :, :],
                    in_ap=ap_of(memd.ap(), 0, [[C, MEMROWS - (WIN - 1)], [1, ELEM]]),
                    idxs_ap=idxw[:, it, l * 32:(l + 1) * 32],
                    num_idxs=512, num_idxs_reg=512,
                    elem_size=ELEM, elem_step=C,
                    queue_num=l % 4)

                prod = work.tile([128, 8 * WIN * WIN, NP], f32, tag="prod")
                for p in range(NP):
                    nc.vector.tensor_mul(
                        ap_of(prod, p, [[1, qlen], [NP, 8 * WIN * WIN]]),
                        ap_of(afy, l * 128 + p * WIN,
                              [[1, qlen], [16, 8], [1, WIN], [0, WIN]]),
                        ap_of(hat, l * 64 + p,
                              [[1, qlen], [4, 8], [0, WIN], [256, WIN]]))
                mmh = work.tile([128, 8 * WIN * WIN, 2], f32, tag="mmh")
                nc.vector.tensor_add(
                    mmh[ql, :, :],
                    ap_of(prod, 0, [[1, qlen], [NP, 8 * WIN * WIN], [1, 2]]),
                    ap_of(prod, 2, [[1, qlen], [NP, 8 * WIN * WIN], [1, 2]]))
                mm = work.tile([128, 8, WIN, WIN], f32, tag="mm")
                nc.vector.tensor_add(
                    mm[ql, :, :, :].rearrange("q h i j -> q (h i j)"),
                    ap_of(mmh, 0, [[1, qlen], [2, 8 * WIN * WIN]]),
                    ap_of(mmh, 1, [[1, qlen], [2, 8 * WIN * WIN]]))
                me16 = work.tile([128, WIN, WIN, 8], bf16, tag="me16")
                nc.scalar.copy(
                    me16[ql, :, :, :],
                    ap_of(mm, 0, [[1, qlen], [4, WIN], [1, WIN], [16, 8]]))
                # winM[q, px, d, h] = win[q, px, (d,h)] * ME16[q, px, h]
                winM = work.tile([128, 4096], bf16, tag="winM")
                nc.vector.tensor_mul(
                    ap_of(winM, 0, [[1, 128], [256, 16], [8, 32], [1, 8]]),
                    ap_of(win, 0, [[1, 128], [256, 16], [8, 32], [1, 8]]),
                    ap_of(me16, 0, [[1, 128], [8, 16], [0, 32], [1, 8]]))
                t1 = work.tile([128, 2048], bf16, tag="t1")
                nc.vector.tensor_add(t1[qf, :], winM[qf, 0:2048],
                                     winM[qf, 2048:4096])
                t2 = work.tile([128, 1024], bf16, tag="t2")
                nc.vector.tensor_add(t2[qf, :], t1[qf, 0:1024], t1[qf, 1024:2048])
                t3 = work.tile([128, 512], bf16, tag="t3")
                nc.vector.tensor_add(t3[qf, :], t2[qf, 0:512], t2[qf, 512:1024])
                nc.vector.tensor_add(res4[qf, l, :], t3[qf, 0:256], t3[qf, 256:512])

            nc.vector.tensor_add(res4[qf, 0:2, :], res4[qf, 0:2, :], res4[qf, 2:4, :])
            res = work.tile([128, 256], bf16, tag="res")
            nc.vector.tensor_add(res[qf, :], res4[qf, 0, :], res4[qf, 1, :])

            resT = work.tile([128, 2, 128], bf16, tag="resT")
            for hh in range(2):
                ps_t = psum_tr.tile([128, 128], bf16, tag="ps_t")
                nc.tensor.transpose(ps_t[:, qf], res[qf, 128 * hh:128 * (hh + 1)],
                                    ident[qf, qf])
                nc.scalar.copy(resT[:, hh, qf], ps_t[:, qf])
            ps_out = psum_o.tile([128, 256], f32, tag="ps_out")
            nc.tensor.matmul(ps_out[qf, :], lhsT=resT[:, 0, qf],
                             rhs=sb_Wout[:, 0, :], start=True, stop=False)
            nc.tensor.matmul(ps_out[qf, :], lhsT=resT[:, 1, qf],
                             rhs=sb_Wout[:, 1, :], start=False, stop=False)
            nc.tensor.matmul(ps_out[qf, :], lhsT=sb_ones[0:1, qf],
                             rhs=sb_bout[0:1, :], start=False, stop=True)
            outt = work.tile([128, 256], f32, tag="outt")
            nc.scalar.copy(outt[ql, :], ps_out[ql, :])
            eng = nc.scalar if it % 2 else nc.sync
            eng.dma_start(out=outd.ap()[q0:q0 + qlen, :], in_=outt[ql, :])

    nc.compile()
    return nc


_NC_CACHE = {}
LAST_RESULTS = None


def _get_nc():
    if "nc" not in _NC_CACHE:
        _NC_CACHE["nc"] = _build_program()
    return _NC_CACHE["nc"]


def host_prep(query, memory, ref_points, W_off, b_off, W_attn, b_attn,
              W_out, b_out):
    """Build the 8 per-core input maps (layout/dtype transforms + gather-
    index addressing; all value compute stays on device)."""
    import ml_dtypes
    bf16 = ml_dtypes.bfloat16

    query = np.ascontiguousarray(query, dtype=np.float32)
    memory = np.ascontiguousarray(memory, dtype=np.float32)
    ref = np.asarray(ref_points, dtype=np.float32)
    W_off = np.asarray(W_off, dtype=np.float32)
    b_off = np.asarray(b_off, dtype=np.float32)
    W_attn = np.asarray(W_attn, dtype=np.float32)
    b_attn = np.asarray(b_attn, dtype=np.float32)
    assert np.all(b_off == 0.0) and np.all(b_attn == 0.0), \
        "nonzero offset/attn biases not folded on device"
    Woff_r = np.ascontiguousarray(
        W_off.reshape(C, NH, NL, NP, 2).transpose(0, 2, 1, 3, 4)
        .reshape(C, 256)).astype(bf16)
    Wattn_r = np.ascontiguousarray(W_attn).astype(bf16)
    Wout_r = np.ascontiguousarray(
        np.asarray(W_out, np.float32).reshape(NH, HD, C)
        .transpose(1, 0, 2).reshape(C, C)).astype(bf16)
    bout = np.ascontiguousarray(
        np.asarray(b_out, dtype=np.float32).reshape(1, C)).astype(bf16)

    wh = np.array([[w, h] for h, w in SPATIAL], dtype=np.float32)
    wh4 = np.array([[w - WIN, h - WIN] for h, w in SPATIAL], np.float32)
    wvec = np.array(W_L, np.float32)
    base = np.array(BASE_L, np.float32)

    in_maps = []
    for c in range(NCORES):
        bs = slice(BPC * c, BPC * (c + 1))
        qT = np.ascontiguousarray(
            query[bs].reshape(QS, C).T).astype(bf16)
        mem = np.ascontiguousarray(
            memory[bs].reshape(MEMROWS, NH, HD).transpose(0, 2, 1)
            .reshape(MEMROWS, C)).astype(bf16)
        refc = ref[bs].reshape(QS, NL, 2)
        refpix = refc * wh[None, :, :] - 0.5
        refpad = np.full((QSP, NL, 2), 5.0, np.float32)
        refpad[:QS] = refpix
        xsc = np.clip(np.floor(refpad) - 1.0, 0.0, wh4[None, :, :])
        pxm = (refpad - xsc).reshape(QSP, NL * 2).astype(np.float32)
        batch = (np.arange(QSP) // Q).clip(max=BPC - 1).astype(np.float32)
        p0 = (xsc[:, :, 1] * wvec[None, :] + xsc[:, :, 0]
              + base[None, :] + batch[:, None] * S)
        idx = (p0[:, :, None] + np.arange(WIN)[None, None, :]
               * wvec[None, :, None]).astype(np.int64)
        idxw = np.empty((128, NQT, NL, 32), np.int16)
        q_of = (np.arange(32) % 8) * 16
        j_of = np.arange(32) // 8
        for t in range(NQT):
            for pp in range(16):
                qq = t * 128 + q_of + pp
                idxw[pp, t, :, :] = idx[qq, :, j_of].transpose(1, 0)
        idxw = np.ascontiguousarray(
            np.broadcast_to(idxw[:16], (8, 16, NQT, NL, 32))
            .reshape(128, NQT * NL * 32))
        in_maps.append(dict(mem=mem, qT=qT, pxm=pxm,
                            idxw=idxw, Woff=Woff_r,
                            Wattn=Wattn_r, Wout=Wout_r, bout=bout))
    return in_maps


def kernel(**inputs):
    global LAST_RESULTS
    from concourse.bass_utils import run_bass_kernel_spmd

    nc = _get_nc()
    in_maps = host_prep(**inputs)
    trace = bool(int(os.environ.get("KERNEL_TRACE", "0")))
    res = run_bass_kernel_spmd(nc, in_maps, core_ids=list(range(NCORES)),
                               trace=trace)
    LAST_RESULTS = res
    out = np.empty((B, Q, C), dtype=np.float32)
    for c in range(NCORES):
        out[BPC * c:BPC * (c + 1)] = res.results[c]["out"].reshape(BPC, Q, C)
    return out
